# revision 1
# baseline (speedup 1.0000x reference)
"""Capsule-routing kernel for Trainium2 (8 NeuronCores, SPMD data-parallel over batch).

Algorithm restructure (hat-free):
  The reference materializes hat = x @ W  (B, N, K*M) = 512 MiB and routes over
  it. All routing contractions reassociate through x and W directly:
    iter 1: c uniform = 1/K  ->  y1 = (1/K) * sum_n x
    s_i  = (c_i^T x) . W_blocks ;  o_i = squash(s_i)
    u_i  = W_blocks . o_i ;  logits_{i+1} = x . (u_1+...+u_i)
  ~8x fewer FLOPs and no 512 MiB hat traffic. Host pre-transposes x per shard
  (xt input) so both (n,d)- and (d,n)-major operands DMA contiguously.

Implementation: raw Block-mode Bass (no Tile scheduler) with hand-placed
semaphores; the program is built as per-engine op lists with sem values
computed inline in global emission order (a topological order, so the wait
graph is deadlock-free by construction). All cross-engine deps are standalone
wait_ge instructions because fp32r matmuls have a single sync-wait slot in
the S3_LW descriptor. Dependent back-to-back DVE ops carry a self-wait
(deep-pipeline retirement hazard, caught by CoreSim).

Matmuls run as float32r (full-rate 4-byte streaming mode, ~11-bit mantissa:
measured end-to-end rel err ~3e-4; KERNEL_F32R=0 falls back to exact fp32 at
4 cycles/row). fp32r outputs must start at psum partition 0, which shapes the
layouts:
  - logits: contraction over (b', d) with a block-diagonal zero-padded
    stationary (ublk), full-128-partition output, (b,k) b-major x N
  - softmax denominator: block-diag-ones matmul = per-group partition
    reduce+broadcast in one op; c normalized with a hw reciprocal
  - c transposed chunk-wise on the PE to (n, (b,k)) for y stationaries;
    y outputs packed at partitions 0-31 into psum banks 2-3
  - yT columns permuted to k-major during psum evacuation so the s-matmul
    diagonal blocks land at free offset 64k; squash runs on the masked
    waste via a per-km-block sumsq reduce (no per-partition compaction)
  - u is built from a second block-diagonal stationary (oblk) scattered via
    PE transposes; compact o is extracted with 32 tiny SBUF->SBUF DMAs only
    for the final store
"""

import os
import numpy as np

NCORES = 8
B_FULL, N, D = 32, 2048, 256
K, M = 32, 64
KM = K * M
BPC = B_FULL // NCORES
EPS = 1e-7
NCHUNK = N // 128   # 16
DCHUNK = D // 128   # 2
USE_F32R = os.environ.get("KERNEL_F32R", "1") == "1"

LAST_EXEC_NS = None
_CACHED = {}


def _build_nc():
    import concourse.bass as bass
    from concourse import mybir

    nc = bass.Bass()
    f32 = mybir.dt.float32
    AF = mybir.ActivationFunctionType
    ALU = mybir.AluOpType

    xp = nc.declare_dram_parameter("xp", [BPC, N, D], f32, isOutput=False)
    xt = nc.declare_dram_parameter("xt", [BPC, D, N], f32, isOutput=False)
    wsb = nc.declare_dram_parameter("wsb", [D, KM], f32, isOutput=False)
    wt = nc.declare_dram_parameter("wt", [KM, D], f32, isOutput=False)
    ctc = nc.declare_dram_parameter("ctc", [128, K], f32, isOutput=False)
    e4 = nc.declare_dram_parameter("e4", [128, 128], f32, isOutput=False)
    id128 = nc.declare_dram_parameter("id128", [128, 128], f32, isOutput=False)
    masks = nc.declare_dram_parameter("masks", [128, 32], f32, isOutput=False)
    zs = nc.declare_dram_parameter("zs", [128, NCHUNK * 128], f32, isOutput=False)
    out = nc.declare_dram_parameter("out", [BPC, K, M], f32, isOutput=True)
    oscr = nc.dram_tensor("oscr", [128, M], f32)

    def r32(ap):
        return ap.bitcast(mybir.dt.float32r) if USE_F32R else ap

    w32 = r32

    # ---- SBUF ----
    x_sb = [nc.alloc_sbuf_tensor(f"x{b}", [128, NCHUNK * D], f32).ap() for b in range(BPC)]
    xt_sb = [
        [nc.alloc_sbuf_tensor(f"xt{b}_{dc}", [128, N], f32).ap() for dc in range(DCHUNK)]
        for b in range(BPC)
    ]
    w_sb = [nc.alloc_sbuf_tensor(f"w{dc}", [128, KM], f32).ap() for dc in range(DCHUNK)]
    wt_sb = nc.alloc_sbuf_tensor("wtt", [128, NCHUNK * D], f32).ap()
    ctc_sb = nc.alloc_sbuf_tensor("ctcs", [128, K], f32).ap()
    e4_sb = nc.alloc_sbuf_tensor("e4s", [128, 128], f32).ap()
    id_sb = nc.alloc_sbuf_tensor("idm", [128, 128], f32).ap()
    msk_sb = nc.alloc_sbuf_tensor("msk", [128, 32], f32).ap()
    oblk = nc.alloc_sbuf_tensor("oblk", [128, NCHUNK * 128], f32).ap()
    ublk = nc.alloc_sbuf_tensor("ublk", [128, 8 * 128], f32).ap()
    expb = nc.alloc_sbuf_tensor("expb", [128, N], f32).ap()
    rden = nc.alloc_sbuf_tensor("rdenc", [128, N], f32).ap()
    cT = nc.alloc_sbuf_tensor("cT", [128, NCHUNK * 128], f32).ap()
    # lifetime-disjoint aliases (SBUF pressure): s-tail scratch reuses expb/cT
    sraw = expb
    sqs = cT
    owst = cT
    y_sb = nc.alloc_sbuf_tensor("ysb", [128, BPC * D], f32).ap()
    yT = nc.alloc_sbuf_tensor("yT", [128, D], f32).ap()
    ssq = nc.alloc_sbuf_tensor("ssq", [128, K], f32).ap()
    
    st = nc.alloc_sbuf_tensor("stt", [128, 8], f32).ap()
    u_sb = nc.alloc_sbuf_tensor("usb", [128, D], f32).ap()
    o_cmp = nc.alloc_sbuf_tensor("ocmp", [128, M], f32).ap()

    # ---- PSUM (8 banks total) ----
    big_ps = nc.alloc_psum_tensor("bigp", [128, 2048], f32).ap()   # 4 banks
    t_ps = [nc.alloc_psum_tensor(f"tp{i}", [128, 128], f32).ap() for i in range(2)]  # 2
    u_ps = nc.alloc_psum_tensor("up", [128, 256], f32).ap()        # 1

    # ---- program construction ----
    ops = {"SP": [], "PE": [], "ACT": [], "DVE": []}
    cnt = {"A": 0, "B": 0, "O": 0, "P": 0, "V": 0, "C": 0,
           "X0": 0, "X1": 0, "X2": 0, "X3": 0, "Z": 0,
           "C1": 0, "I1": 0, "M1": 0, "W1": 0, "T1": 0}
    waited = {e: {} for e in ops}

    def emit(eng, fn, waits=(), inc=None, inc_by=1):
        waits = list(waits)
        if eng == "DVE":
            # DVE pipeline: dependent back-to-back DVE ops need retirement
            # ordering; self-wait on the previous op's sem value.
            waits.append(("V", cnt["V"]))
        real = []
        for sem_key, val in waits:
            if val > 0 and waited[eng].get(sem_key, -1) < val:
                real.append((sem_key, val))
                waited[eng][sem_key] = val
        ops[eng].append((tuple(real), fn, inc, inc_by))
        if inc is not None:
            cnt[inc] += inc_by

    def pe_mm(outap, lhsT, rhs, start, stop, tp=None, waits=()):
        emit("PE",
             lambda o=outap, l=lhsT, r=rhs, s=start, t=stop,
             p=(tp if tp is not None and tp[1] == 96 else None):
             nc.tensor.matmul(o, r32(l), r32(r), start=s, stop=t, tile_position=p),
             waits=waits, inc="P")

    def pe_tr(outap, inap, waits=()):
        waits = list(waits) + [("I1", 16)]
        p = inap.partition_size()
        emit("PE",
             lambda o=outap, i=inap, p=p: nc.tensor.transpose(o, i, id_sb[0:p, 0:p]),
             waits=waits, inc="P")

    # ---- loads ----
    def dma(outap, inap, grp, waits=(), eng="SP"):
        dge = nc.sync if eng == "SP" else nc.scalar
        emit(eng, lambda o=outap, i=inap, g=dge: g.dma_start(out=o, in_=i),
             waits=waits, inc=grp, inc_by=16)

    dma(w32(ctc_sb), w32(ctc[:]), "C1")
    dma(w32(id_sb), w32(id128[:]), "I1")
    dma(msk_sb, masks[:], "M1")
    for dc in range(DCHUNK):
        dma(w32(w_sb[dc]), w32(wsb[dc * 128:(dc + 1) * 128, :]), "W1")
    dma(w32(wt_sb.rearrange("p (t d) -> p t d", d=D)),
        w32(wt.rearrange("(t p) d -> p t d", p=128)), "T1")
    for b in range(BPC):
        dma(w32(x_sb[b].rearrange("p (j d) -> p j d", d=D)),
            w32(xp[b].rearrange("(j p) d -> p j d", p=128)), f"X{b}",
            eng="SP")
    dma(w32(oblk), w32(zs[:]), "Z")
    dma(w32(ublk), w32(zs[:, 0:8 * 128]), "Z")
    ZV = cnt["Z"]
    dma(w32(e4_sb), w32(e4[:]), "B")
    for b in range(BPC):
        for dc in range(DCHUNK):
            dma(w32(xt_sb[b][dc]), w32(xt[b, dc * 128:(dc + 1) * 128, :]), "B",
                eng="SP")
    B_FULL_V = cnt["B"]


    # ---- iterations ----
    ublk_v = {}
    for it in range(3):
        if it == 0:
            for b in range(BPC):
                for j in range(NCHUNK):
                    pe_mm(big_ps[0:K, 1024 + b * D: 1024 + (b + 1) * D], ctc_sb,
                          x_sb[b][:, j * D:(j + 1) * D],
                          start=(j == 0), stop=(j == NCHUNK - 1),
                          waits=[("C1", 16), (f"X{b}", 16)])
        else:
            # logits: contraction over (b', d) with block-diag zero-padded ublk
            btv = []
            for sl in range(4):
                for ch in range(8):
                    pe_mm(big_ps[:, sl * 512:(sl + 1) * 512],
                          ublk[:, ch * 128:(ch + 1) * 128],
                          xt_sb[ch // 2][ch % 2][:, sl * 512:(sl + 1) * 512],
                          start=(ch == 0), stop=(ch == 7),
                          waits=[("B", B_FULL_V), ("V", ublk_v[ch])])
                btv.append(cnt["P"])
            # softmax: exp per slice overlaps the remaining logit matmuls
            expv = []
            for sl in range(4):
                emit("ACT",
                     lambda sl=sl: nc.scalar.activation(
                         w32(expb[:, sl * 512:(sl + 1) * 512]),
                         big_ps[:, sl * 512:(sl + 1) * 512], AF.Exp),
                     waits=[("P", btv[sl])], inc="C")
                expv.append(cnt["C"])
            for sl in range(4):
                pe_mm(big_ps[:, sl * 512:(sl + 1) * 512], e4_sb,
                      expb[:, sl * 512:(sl + 1) * 512],
                      start=True, stop=True, waits=[("C", expv[sl])])
            emit("DVE", lambda: nc.vector.reciprocal(rden, big_ps),
                 waits=[("P", cnt["P"])], inc="V")
            emit("DVE", lambda: nc.vector.tensor_mul(rden, expb, rden), inc="V")
            c_sb = rden
            # cT transposes (ping-pong psum slots) then y matmuls
            copy_v = {}
            for j in range(NCHUNK):
                w = [("V", copy_v[j - 2])] if j >= 2 else [("V", cnt["V"])]
                pe_tr(t_ps[j % 2], c_sb[:, j * 128:(j + 1) * 128], waits=w)
                trp = cnt["P"]
                emit("DVE",
                     lambda j=j: nc.vector.tensor_copy(
                         w32(cT[:, j * 128:(j + 1) * 128]), t_ps[j % 2]),
                     waits=[("P", trp)], inc="V")
                copy_v[j] = cnt["V"]
            for b in range(BPC):
                for j in range(NCHUNK):
                    pe_mm(big_ps[0:K, 1024 + b * D: 1024 + (b + 1) * D],
                          cT[:, j * 128 + b * K: j * 128 + (b + 1) * K],
                          x_sb[b][:, j * D:(j + 1) * D],
                          start=(j == 0), stop=(j == NCHUNK - 1),
                          waits=[("V", copy_v[j])])

        # ---- shared tail ----
        emit("ACT", lambda: nc.scalar.activation(
                 y_sb[0:K, :], big_ps[0:K, 1024:2048], AF.Copy),
             waits=[("P", cnt["P"])], inc="C")
        yev = cnt["C"]
        ycp_v = {}
        for b in range(BPC):
            for dc in range(DCHUNK):
                i = b * DCHUNK + dc
                w = [("V", ycp_v[i - 2])] if i >= 2 else [("C", yev), ("V", cnt["V"])]
                pe_tr(t_ps[i % 2][:, 0:K],
                      y_sb[0:K, b * D + dc * 128: b * D + (dc + 1) * 128],
                      waits=w)
                trp = cnt["P"]
                emit("DVE",
                     lambda b=b, dc=dc, i=i: nc.vector.tensor_copy(
                         w32(yT[:, dc * 128:(dc + 1) * 128]
                             .rearrange("d (k g) -> d k g", g=BPC)[:, :, b]),
                         t_ps[i % 2][:, 0:K]),
                     waits=[("P", trp)], inc="V")
                ycp_v[i] = cnt["V"]
        wv = cnt["V"]
        for q in range(4):
            for dc in range(DCHUNK):
                pe_mm(big_ps[:, 512 * q:512 * (q + 1)],
                      yT[:, dc * 128:(dc + 1) * 128],
                      w_sb[dc][:, 512 * q:512 * (q + 1)],
                      start=(dc == 0), stop=(dc == DCHUNK - 1),
                      waits=[("V", wv), ("W1", 32)])
        emit("ACT", lambda: nc.scalar.activation(w32(sraw), big_ps, AF.Copy),
             waits=[("P", cnt["P"])], inc="C")
        emit("DVE", lambda: nc.vector.tensor_mul(w32(sqs), sraw, sraw),
             waits=[("C", cnt["C"])], inc="V")
        emit("DVE",
             lambda: nc.vector.tensor_reduce(
                 ssq, sqs.rearrange("p (k m) -> p k m", m=M),
                 axis=mybir.AxisListType.X, op=ALU.add),
             inc="V")
        emit("DVE", lambda: nc.vector.tensor_mul(ssq, ssq, msk_sb),
             waits=[("M1", 16)], inc="V")
        emit("DVE",
             lambda: nc.vector.tensor_reduce(
                 st[:, 0:1], ssq, axis=mybir.AxisListType.X, op=ALU.add),
             inc="V")
        emit("DVE", lambda: nc.vector.tensor_scalar_add(st[:, 5:6], st[:, 0:1], EPS),
             inc="V")
        emit("ACT", lambda: nc.scalar.activation(st[:, 1:2], st[:, 5:6], AF.Sqrt),
             waits=[("V", cnt["V"])], inc="C")
        emit("DVE",
             lambda: nc.vector.tensor_scalar_add(st[:, 2:3], st[:, 0:1], 0.5 + EPS),
             waits=[("C", cnt["C"])], inc="V")
        emit("DVE", lambda: nc.vector.reciprocal(st[:, 3:4], st[:, 2:3]), inc="V")
        emit("DVE", lambda: nc.vector.tensor_mul(st[:, 4:5], st[:, 1:2], st[:, 3:4]),
             inc="V")
        emit("DVE",
             lambda: nc.vector.tensor_scalar(w32(owst), sraw, st[:, 4:5], None, ALU.mult),
             inc="V")
        owst_v = cnt["V"]

        if it == 2:
            for k in range(K):
                emit("SP",
                     lambda k=k: nc.sync.dma_start(
                         out=o_cmp[4 * k:4 * (k + 1), :],
                         in_=owst[4 * k:4 * (k + 1), 64 * k:64 * k + 64]),
                     waits=[("V", owst_v)], inc="O", inc_by=16)
            ofirst = cnt["O"]
            emit("SP",
                 lambda: nc.sync.dma_start(out=oscr[:], in_=o_cmp),
                 waits=[("O", ofirst)], inc="O", inc_by=16)
            osecond = cnt["O"]
            emit("SP",
                 lambda: nc.sync.dma_start(
                     out=out.rearrange("b k m -> k b m"),
                     in_=oscr.rearrange("(k b) m -> k b m", b=BPC)),
                 waits=[("O", osecond)], inc="O", inc_by=16)
            emit("SP", lambda: None, waits=[("O", cnt["O"])])
            continue

        # u-step
        scat_v = {}
        for t in range(NCHUNK):
            w = [("V", scat_v[t - 2])] if t >= 2 else [("V", owst_v)]
            pe_tr(t_ps[t % 2], owst[:, 128 * t:128 * (t + 1)], waits=w)
            trp = cnt["P"]
            ke, ko = 2 * t, 2 * t + 1
            emit("DVE",
                 lambda t=t, ke=ke: nc.vector.tensor_copy(
                     w32(oblk[0:64, 128 * t + 4 * ke: 128 * t + 4 * ke + BPC]),
                     t_ps[t % 2][0:64, 4 * ke: 4 * ke + BPC]),
                 waits=[("P", trp), ("Z", ZV)], inc="V")
            emit("DVE",
                 lambda t=t, ko=ko: nc.vector.tensor_copy(
                     w32(oblk[64:128, 128 * t + 4 * ko: 128 * t + 4 * ko + BPC]),
                     t_ps[t % 2][64:128, 4 * ko: 4 * ko + BPC]),
                 inc="V")
            scat_v[t] = cnt["V"]
        for t in range(NCHUNK):
            pe_mm(u_ps, oblk[:, t * 128:(t + 1) * 128], wt_sb[:, t * D:(t + 1) * D],
                  start=(t == 0), stop=(t == NCHUNK - 1),
                  waits=[("V", scat_v[t]), ("T1", 16)])
        emit("DVE", lambda: nc.vector.tensor_copy(u_sb, u_ps),
             waits=[("P", cnt["P"])], inc="V")
        wv = cnt["V"]
        utr_p = []
        for dc in range(DCHUNK):
            pe_tr(t_ps[dc], u_sb[:, dc * 128:(dc + 1) * 128], waits=[("V", wv)])
            utr_p.append(cnt["P"])
        # scatter uT chunks into block-diag ublk (cols b-major within chunk)
        ublk_v.clear()
        for dc in range(DCHUNK):
            for b in range(BPC):
                ch = 2 * b + dc
                uv = ublk[:, ch * 128 + b * K: ch * 128 + (b + 1) * K]
                tv = t_ps[dc].rearrange("d (k g) -> d g k", g=BPC)[:, b, :]
                if it == 0:
                    emit("DVE", lambda uv=uv, tv=tv: nc.vector.tensor_copy(w32(uv), tv),
                         waits=[("P", utr_p[dc]), ("Z", ZV)], inc="V")
                else:
                    emit("DVE", lambda uv=uv, tv=tv: nc.vector.tensor_add(w32(uv), uv, tv),
                         waits=[("P", utr_p[dc])], inc="V")
                ublk_v[ch] = cnt["V"]

    # ---- emission: one body per engine ----
    with (
        nc.semaphore("sA") as sA,
        nc.semaphore("sB") as sB,
        nc.semaphore("sO") as sO,
        nc.semaphore("sP") as sP,
        nc.semaphore("sV") as sV,
        nc.semaphore("sC") as sC,
        nc.semaphore("sX0") as sX0,
        nc.semaphore("sX1") as sX1,
        nc.semaphore("sX2") as sX2,
        nc.semaphore("sX3") as sX3,
        nc.semaphore("sZ") as sZ,
        nc.semaphore("sC1") as sC1,
        nc.semaphore("sI1") as sI1,
        nc.semaphore("sM1") as sM1,
        nc.semaphore("sW1") as sW1,
        nc.semaphore("sT1") as sT1,
        nc.Block() as block,
    ):
        sem_handles = {"A": sA, "B": sB, "O": sO, "P": sP, "V": sV, "C": sC,
                       "X0": sX0, "X1": sX1, "X2": sX2, "X3": sX3, "Z": sZ,
                       "C1": sC1, "I1": sI1, "M1": sM1, "W1": sW1, "T1": sT1}

        def run_ops(eng_name):
            def body(e):
                for waits, fn, inc, inc_by in ops[eng_name]:
                    for sem_key, val in waits:
                        e.wait_ge(sem_handles[sem_key], val)
                    inst = fn()
                    if inc is not None and inst is not None:
                        inst.then_inc(sem_handles[inc], inc_by)
            return body

        block.sync(run_ops("SP"))
        block.tensor(run_ops("PE"))
        block.scalar(run_ops("ACT"))
        block.vector(run_ops("DVE"))
    return nc


def _get_nc():
    if "nc" not in _CACHED:
        _CACHED["nc"] = _build_nc()
    return _CACHED["nc"]


def kernel(x, W):
    global LAST_EXEC_NS
    from concourse.bass_utils import run_bass_kernel_spmd

    x = np.ascontiguousarray(x, dtype=np.float32)
    W = np.ascontiguousarray(W, dtype=np.float32)
    assert x.shape == (B_FULL, N, D) and W.shape == (D, KM)

    nc = _get_nc()

    ctc = np.full((128, K), 1.0 / K, dtype=np.float32)
    e4 = np.kron(np.eye(BPC, dtype=np.float32), np.ones((K, K), dtype=np.float32))
    id128 = np.eye(128, dtype=np.float32)
    wt = np.ascontiguousarray(W.T)
    masks = np.zeros((128, 32), dtype=np.float32)
    for p in range(128):
        masks[p, p // BPC] = 1.0
    zs = np.zeros((128, 2048), dtype=np.float32)

    in_maps = []
    for i in range(NCORES):
        xs = np.ascontiguousarray(x[i * BPC:(i + 1) * BPC])
        xts = np.ascontiguousarray(xs.transpose(0, 2, 1))
        in_maps.append(
            {"xp": xs, "xt": xts, "wsb": W, "wt": wt, "ctc": ctc, "e4": e4,
             "id128": id128, "masks": masks, "zs": zs}
        )

    trace = os.environ.get("KERNEL_TRACE", "0") == "1"
    res = run_bass_kernel_spmd(nc, in_maps, list(range(NCORES)), trace=trace)
    LAST_EXEC_NS = res.exec_time_ns
    outs = [res.results[i]["out"] for i in range(NCORES)]
    return np.concatenate(outs, axis=0)



# revision 9
# speedup vs baseline: 2.2650x; 2.2650x over previous
"""Capsule-routing kernel for Trainium2 (8 NeuronCores, SPMD data-parallel over batch).

Hat-free routing (see reference): per iteration
    l = x.u  (logits)  ->  c = softmax_k(l)  ->  y = c.x  ->  s_k = y_k W_k
    o = squash(s)      ->  u_k = W_k o_k     ->  accumulate u for next l

All matmuls are shaped so the *output free size* is small and the moving
operand is bf16 (full-rate at any width):
  - logits are produced n-PARTITIONED: lT[n, (b,j,k)] tiles [128, 32]
    (stationary = xT chunk, moving = uT[d, k] slice). Softmax over k is then
    a free-dim grouped reduce + broadcast divide - no transposes, and the
    resulting cT[n, (b,j,k)] slices feed the y matmuls directly as moving.
  - y is produced d-partitioned: yT[d, k] tiles [128, 32] (stationary =
    x chunk), landing pre-transposed for the s matmul stationary.
  - s = yT.W in [128(k,b), 2048] (diag-block layout), evacuated per-slice.
  - u via block-diag oblk built from 16 PE transposes of s + 8 paired
    masked-multiplies (two 8-col windows at stride 136 in one DVE op).
  - squash scale is applied at u/output evacuation (per-partition (k,b)
    scalar), keeping the squash scalar chain off the u critical path.
Work is spread over PE / ACT / DVE / GPSIMD(Pool): exp + evacuations on ACT,
softmax normalize on Pool, reduces + scatters on DVE.

Raw Block-mode Bass with hand-placed semaphores; ops are emitted per-engine
in one global topological order with sem values computed inline.
"""

import os
import numpy as np

NCORES = 8
B_FULL, N, D = 32, 2048, 256
K, M = 32, 64
KM = K * M
BPC = B_FULL // NCORES
EPS = 1e-7
NCHUNK = N // 128   # 16
DCHUNK = D // 128   # 2

LAST_EXEC_NS = None
_CACHED = {}


def _build_nc():
    import concourse.bass as bass
    from concourse import mybir
    from concourse.alu_op_type import AluOpType as ALU
    from concourse.ap import AP as BassAP

    nc = bass.Bass()
    f32 = mybir.dt.float32
    bf16 = mybir.dt.bfloat16
    AF = mybir.ActivationFunctionType
    AX = mybir.AxisListType

    # ---- DRAM ----
    xb = nc.declare_dram_parameter("xb", [BPC, N, D], bf16, isOutput=False)
    xtb = nc.declare_dram_parameter("xtb", [BPC, D, N], bf16, isOutput=False)
    wb = nc.declare_dram_parameter("wb", [D, KM], bf16, isOutput=False)
    wtb = nc.declare_dram_parameter("wtb", [KM, D], bf16, isOutput=False)
    cc0 = nc.declare_dram_parameter("cc0", [128, K], bf16, isOutput=False)
    idb = nc.declare_dram_parameter("idb", [128, 128], bf16, isOutput=False)
    msk8 = nc.declare_dram_parameter("msk8", [128, 8], bf16, isOutput=False)
    msk32 = nc.declare_dram_parameter("msk32", [128, K], bf16, isOutput=False)
    out = nc.declare_dram_parameter("out", [BPC, K, M], f32, isOutput=True)

    # ---- SBUF ----
    x_sb = [nc.alloc_sbuf_tensor(f"x{b}", [128, NCHUNK * D], bf16).ap() for b in range(BPC)]
    xt_sb = [
        [nc.alloc_sbuf_tensor(f"xt{b}_{dc}", [128, N], bf16).ap() for dc in range(DCHUNK)]
        for b in range(BPC)
    ]
    w_sb = [nc.alloc_sbuf_tensor(f"w{dc}", [128, KM], bf16).ap() for dc in range(DCHUNK)]
    wt_sb = nc.alloc_sbuf_tensor("wtt", [128, NCHUNK * D], bf16).ap()
    cc0_sb = nc.alloc_sbuf_tensor("cc0s", [128, K], bf16).ap()
    idb_sb = nc.alloc_sbuf_tensor("idbs", [128, 128], bf16).ap()
    m8_sb = nc.alloc_sbuf_tensor("m8s", [128, 8], bf16).ap()
    m32_sb = nc.alloc_sbuf_tensor("m32s", [128, K], bf16).ap()
    et = nc.alloc_sbuf_tensor("et", [128, N], bf16).ap()
    den = nc.alloc_sbuf_tensor("den", [128, 64], bf16).ap()
    rden = nc.alloc_sbuf_tensor("rden", [128, 64], bf16).ap()
    cT = nc.alloc_sbuf_tensor("cT", [128, N], bf16).ap()
    yT_sb = [nc.alloc_sbuf_tensor(f"yT{dc}", [128, 128], bf16).ap() for dc in range(DCHUNK)]
    sraw = nc.alloc_sbuf_tensor("sraw", [128, KM], bf16).ap()
    sq = nc.alloc_sbuf_tensor("sq", [128, KM], bf16).ap()
    ssq32 = nc.alloc_sbuf_tensor("ssq32", [128, K], bf16).ap()
    ssqm = nc.alloc_sbuf_tensor("ssqm", [128, K], bf16).ap()
    st = nc.alloc_sbuf_tensor("stt", [128, 8], f32).ap()
    u_sb = nc.alloc_sbuf_tensor("usb", [128, D], bf16).ap()
    uT_sb = [nc.alloc_sbuf_tensor(f"uT{dc}", [128, 128], bf16).ap() for dc in range(DCHUNK)]
    # padded +128 cols so the last masked pair-window view stays in range
    oblk = nc.alloc_sbuf_tensor("oblk", [128, NCHUNK * 128 + 128], bf16).ap()
    owst = nc.alloc_sbuf_tensor("owst", [128, KM], f32).ap()
    oscr = nc.dram_tensor("oscr", [128, KM], f32)

    # ---- PSUM (8 banks) ----
    big_ps = nc.alloc_psum_tensor("bigp", [128, 2048], f32).ap()      # banks 0-3
    tq = [nc.alloc_psum_tensor(f"tq{i}", [128, 1024], bf16).ap() for i in range(2)]  # 4,5
    yT_ps = nc.alloc_psum_tensor("yTp", [128, 256], f32).ap()         # bank 6
    u_ps = nc.alloc_psum_tensor("up", [128, 256], f32).ap()           # bank 7

    # ---- program construction ----
    ops = {"SP": [], "PE": [], "ACT": [], "DVE": [], "POOL": []}
    cnt = {"P": 0, "V": 0, "C": 0, "L": 0, "O": 0,
           "X0": 0, "X1": 0, "X2": 0, "X3": 0,
           "T0": 0, "T1": 0, "T2": 0, "T3": 0,
           "W1": 0, "W2": 0, "M1": 0}
    waited = {e: {} for e in ops}

    def emit(eng, fn, waits=(), inc=None, inc_by=1):
        best = {}
        for sem_key, val in waits:
            if val > 0 and best.get(sem_key, -1) < val:
                best[sem_key] = val
        real = []
        for sem_key, val in best.items():
            if waited[eng].get(sem_key, -1) < val:
                real.append((sem_key, val))
                waited[eng][sem_key] = val
        ops[eng].append((tuple(real), fn, inc, inc_by))
        if inc is not None:
            cnt[inc] += inc_by

    def pe_mm(outap, lhsT, rhs, start, stop, waits=()):
        emit("PE",
             lambda o=outap, l=lhsT, r=rhs, s=start, t=stop:
             nc.tensor.matmul(o, l, r, start=s, stop=t),
             waits=waits, inc="P")

    def pe_tr(outap, inap, waits=()):
        waits = list(waits) + [("M1", 64)]
        emit("PE",
             lambda o=outap, i=inap: nc.tensor.transpose(o, i, idb_sb),
             waits=waits, inc="P")

    def dma(outap, inap, grp, waits=()):
        emit("SP", lambda o=outap, i=inap: nc.sync.dma_start(out=o, in_=i),
             waits=waits, inc=grp, inc_by=16)

    # ---- loads (x first: iter0 is x-DMA-paced) ----
    for b in range(BPC):
        dma(x_sb[b].rearrange("p (j d) -> p j d", d=D),
            xb[b].rearrange("(j p) d -> p j d", p=128), f"X{b}")
    dma(cc0_sb, cc0[:], "M1")
    dma(idb_sb, idb[:], "M1")
    dma(m8_sb, msk8[:], "M1")
    dma(m32_sb, msk32[:], "M1")
    for dc in range(DCHUNK):
        dma(w_sb[dc], wb[dc * 128:(dc + 1) * 128, :], "W1")
    dma(wt_sb.rearrange("p (t d) -> p t d", d=D),
        wtb.rearrange("(t p) d -> p t d", p=128), "W2")
    for b in range(BPC):
        for dc in range(DCHUNK):
            dma(xt_sb[b][dc], xtb[b, dc * 128:(dc + 1) * 128, :], f"T{b}")

    # zero-fill oblk once; masked writes refresh only the valid windows
    emit("DVE", lambda: nc.vector.memset(oblk, 0.0), inc="V")
    MEMSET_L = ("V", cnt["V"])

    # cross-iteration state (sem values of producers/consumers)
    sevac_v = [0] * 4          # ACT sraw evac per slice (C)
    sq_v = [0] * 4             # DVE sq per slice (V)
    tr_last_v = [0] * 4        # PE last sraw-transpose reading slice q (P)
    expv = [0] * 4             # ACT exp per slice (C)
    cTv = [0] * 4              # POOL cT normalize per slice (L)
    yuse_v = [0] * 4           # PE last y-mm reading cT slice b (P)
    ydc_v = [0] * 2            # PE y mms done per dc (P)
    yev_v = [0] * 2            # ACT yT evac per dc (C)
    utev_v = [0] * 2           # DVE uT evac per dc (V)
    utr_v = [0] * 2            # PE u transposes (P)
    uall_v = 0                 # PE u mms done (P)
    uevac_v = 0                # DVE u evac (V)
    logits_v = 0               # PE logits done (P)
    logits_done = [0] * 4

    for it in range(3):
        # ---- A/B: logits + softmax (it > 0) ----
        # NOTE: a psum tensor must have NO open accumulation groups when any
        # engine reads it, so exp waits for ALL logits tiles to stop.
        if it > 0:
            for b in range(BPC):
                for j in range(NCHUNK):
                    g = b * NCHUNK + j
                    for dc in range(DCHUNK):
                        pe_mm(big_ps[:, g * 32:(g + 1) * 32],
                              xt_sb[b][dc][:, j * 128:(j + 1) * 128],
                              uT_sb[dc][:, b * 32:(b + 1) * 32],
                              start=(dc == 0), stop=(dc == 1),
                              waits=[(f"T{b}", 32),
                                     ("V", max(utev_v)),
                                     ("C", sevac_v[3])])
            logits_v = cnt["P"]
            for b in range(BPC):
                sl = slice(b * 512, (b + 1) * 512)
                gsl = slice(b * 16, (b + 1) * 16)
                emit("ACT",
                     lambda sl=sl: nc.scalar.activation(et[:, sl], big_ps[:, sl], AF.Exp),
                     waits=[("P", logits_v), ("V", cTv[b])], inc="C")
                expv[b] = cnt["C"]
                emit("DVE",
                     lambda sl=sl, gsl=gsl: nc.vector.tensor_reduce(
                         den[:, gsl], et[:, sl].rearrange("p (g k) -> p g k", k=K),
                         AX.X, ALU.add),
                     waits=[("C", expv[b])], inc="V")
                dv = cnt["V"]
                emit("DVE",
                     lambda gsl=gsl: nc.vector.reciprocal(rden[:, gsl], den[:, gsl]),
                     waits=[("V", dv)], inc="V")
                rv = cnt["V"]
                emit("DVE",
                     lambda sl=sl, gsl=gsl: nc.vector.tensor_tensor(
                         cT[:, sl].rearrange("p (g k) -> p g k", k=K),
                         et[:, sl].rearrange("p (g k) -> p g k", k=K),
                         rden[:, gsl].rearrange("p (g o) -> p g o", o=1)
                         .broadcast_to([128, 16, K]),
                         ALU.mult),
                     waits=[("V", rv), ("P", yuse_v[b])], inc="V")
                cTv[b] = cnt["V"]

        # ---- C: y matmuls -> yT tiles [128(d), 32(k)] per (b, dc) ----
        for dc in range(DCHUNK):
            for b in range(BPC):
                for j in range(NCHUNK):
                    mov = (cc0_sb if it == 0
                           else cT[:, (b * NCHUNK + j) * 32:(b * NCHUNK + j + 1) * 32])
                    w = []
                    if it == 0:
                        w = [(f"X{b}", 16), ("M1", 64)]
                    else:
                        w = [("V", cTv[b])]
                    w.append(("C", yev_v[dc]))
                    pe_mm(yT_ps[:, dc * 128 + b * 32: dc * 128 + (b + 1) * 32],
                          x_sb[b][:, j * D + dc * 128: j * D + (dc + 1) * 128],
                          mov, start=(j == 0), stop=(j == NCHUNK - 1), waits=w)
                yuse_v[b] = cnt["P"]
            ydc_v[dc] = cnt["P"]

        # ---- D: yT evac (ACT): permute (b,k) -> (k,b) cols, f32 -> bf16 ----
        for dc in range(DCHUNK):
            emit("ACT",
                 lambda dc=dc: nc.scalar.activation(
                     yT_sb[dc].rearrange("p (k g) -> p g k", g=BPC),
                     yT_ps[:, dc * 128:(dc + 1) * 128]
                     .rearrange("p (g k) -> p g k", k=K),
                     AF.Copy),
                 waits=[("P", max(ydc_v))], inc="C")
            yev_v[dc] = cnt["C"]

        # ---- E: s matmuls [128(k,b), 2048] ----
        for q in range(4):
            for dc in range(DCHUNK):
                w = [("C", yev_v[dc]), ("W1", 32)]
                if it > 0:
                    w.append(("C", expv[q]))
                pe_mm(big_ps[:, q * 512:(q + 1) * 512], yT_sb[dc],
                      w_sb[dc][:, q * 512:(q + 1) * 512],
                      start=(dc == 0), stop=(dc == 1), waits=w)
        s_all = cnt["P"]
        for q in range(4):
            # ---- F: per-slice evac + sumsq ----
            emit("ACT",
                 lambda q=q: nc.scalar.activation(
                     sraw[:, q * 512:(q + 1) * 512],
                     big_ps[:, q * 512:(q + 1) * 512], AF.Copy),
                 waits=[("P", s_all), ("V", sq_v[q]), ("P", tr_last_v[q])], inc="C")
            sevac_v[q] = cnt["C"]
            emit("DVE",
                 lambda q=q: nc.vector.tensor_tensor(
                     sq[:, q * 512:(q + 1) * 512],
                     sraw[:, q * 512:(q + 1) * 512],
                     sraw[:, q * 512:(q + 1) * 512], ALU.mult),
                 waits=[("C", sevac_v[q])], inc="V")
            sq_v[q] = cnt["V"]
            emit("DVE",
                 lambda q=q: nc.vector.tensor_reduce(
                     ssq32[:, q * 8:(q + 1) * 8],
                     sq[:, q * 512:(q + 1) * 512].rearrange("p (k m) -> p k m", m=M),
                     AX.X, ALU.add),
                 waits=[("V", sq_v[q])], inc="V")

        # ---- G: squash scalars ----
        emit("DVE", lambda: nc.vector.tensor_tensor(ssqm, ssq32, m32_sb, ALU.mult),
             waits=[("M1", 64), ("V", cnt["V"])], inc="V")
        mk_v = cnt["V"]
        emit("DVE", lambda: nc.vector.tensor_reduce(st[:, 0:1], ssqm, AX.X, ALU.add),
             waits=[("V", mk_v)], inc="V")
        tot_v = cnt["V"]
        emit("DVE", lambda: nc.vector.tensor_scalar_add(st[:, 5:6], st[:, 0:1], EPS),
             waits=[("V", tot_v)], inc="V")
        eps_v = cnt["V"]
        emit("ACT", lambda: nc.scalar.activation(st[:, 1:2], st[:, 5:6], AF.Sqrt),
             waits=[("V", eps_v)], inc="C")
        sqrt_v = cnt["C"]
        emit("DVE", lambda: nc.vector.tensor_scalar_add(st[:, 2:3], st[:, 0:1],
                                                        0.5 + EPS),
             waits=[("V", tot_v)], inc="V")
        st2_v = cnt["V"]

        if it < 2:
            # ---- H: u path ----
            # 16 transposes of sraw chunks into paired psum slots; 8 masked
            # pair-multiplies scatter the diagonal blocks into oblk.
            mp_v = [0] * 8
            for p in range(8):
                for s in range(2):
                    t = 2 * p + s
                    w = [("C", sevac_v[t // 4])]
                    if p >= 2:
                        w.append(("V", mp_v[p - 2]))
                    if p < 2:
                        # slots last read by prev-iter uT evac / masked ops
                        w.append(("V", max(utev_v)))
                    pe_tr(tq[p % 2][:, s * 128:(s + 1) * 128],
                          sraw[:, t * 128:(t + 1) * 128], waits=w)
                    tr_last_v[t // 4] = cnt["P"]
                trp = cnt["P"]
                emit("DVE",
                     lambda p=p: nc.vector.tensor_tensor(
                         oblk[:, 272 * p: 272 * p + 272]
                         .rearrange("d (s r) -> d s r", s=2)[:, :, 0:8],
                         tq[p % 2][:, 16 * p: 16 * p + 272]
                         .rearrange("d (s r) -> d s r", s=2)[:, :, 0:8],
                         m8_sb.rearrange("d (o r) -> d o r", o=1)
                         .broadcast_to([128, 2, 8]),
                         ALU.mult),
                     waits=[("P", trp), ("M1", 64), MEMSET_L,
                            ("P", uall_v)], inc="V")
                mp_v[p] = cnt["V"]
            for t in range(NCHUNK):
                pe_mm(u_ps, oblk[:, t * 128:(t + 1) * 128],
                      wt_sb[:, t * D:(t + 1) * D],
                      start=(t == 0), stop=(t == NCHUNK - 1),
                      waits=[("V", mp_v[t // 2]), ("W2", 16), ("V", uevac_v)])
            uall_v = cnt["P"]
            # scale chain: scale = sqrt(ssq+eps) / (0.5+eps+ssq)
            emit("DVE", lambda: nc.vector.reciprocal(st[:, 3:4], st[:, 2:3]),
                 waits=[("V", st2_v)], inc="V")
            emit("DVE", lambda: nc.vector.tensor_mul(st[:, 4:5], st[:, 1:2],
                                                     st[:, 3:4]),
                 waits=[("C", sqrt_v), ("V", cnt["V"])], inc="V")
            scale_v = cnt["V"]
            emit("DVE",
                 lambda: nc.vector.tensor_scalar(u_sb, u_ps, st[:, 4:5], None,
                                                 ALU.mult),
                 waits=[("P", uall_v), ("V", scale_v), ("P", utr_v[1])], inc="V")
            uevac_v = cnt["V"]
            for dc in range(DCHUNK):
                pe_tr(tq[dc][:, 0:128], u_sb[:, dc * 128:(dc + 1) * 128],
                      waits=[("V", uevac_v), ("V", mp_v[6 + dc])])
                utr_v[dc] = cnt["P"]
                if it == 0:
                    emit("DVE",
                         lambda dc=dc: nc.vector.tensor_copy(
                             uT_sb[dc].rearrange("p (g k) -> p k g", k=K),
                             tq[dc][:, 0:128].rearrange("p (k g) -> p k g", g=BPC)),
                         waits=[("P", utr_v[dc])], inc="V")
                else:
                    emit("DVE",
                         lambda dc=dc: nc.vector.tensor_add(
                             uT_sb[dc].rearrange("p (g k) -> p k g", k=K),
                             uT_sb[dc].rearrange("p (g k) -> p k g", k=K),
                             tq[dc][:, 0:128].rearrange("p (k g) -> p k g", g=BPC)),
                         waits=[("P", utr_v[dc]), ("P", logits_v)], inc="V")
                utev_v[dc] = cnt["V"]
        else:
            # ---- I: final output ----
            # DVE/ACT partition base must be 0/32/64/96, so the diagonal
            # blocks cannot be extracted with 4-partition tensor ops. Instead:
            # full-width owst = scale*s (f32), bounce through a DRAM scratch,
            # then one diagonal-gather DMA (manual AP) into the output.
            emit("DVE", lambda: nc.vector.reciprocal(st[:, 3:4], st[:, 2:3]),
                 waits=[("V", st2_v)], inc="V")
            emit("DVE", lambda: nc.vector.tensor_mul(st[:, 4:5], st[:, 1:2],
                                                     st[:, 3:4]),
                 waits=[("C", sqrt_v), ("V", cnt["V"])], inc="V")
            scale_v = cnt["V"]
            for q in range(4):
                emit("DVE",
                     lambda q=q: nc.vector.tensor_scalar(
                         owst[:, q * 512:(q + 1) * 512],
                         sraw[:, q * 512:(q + 1) * 512],
                         st[:, 4:5], None, ALU.mult),
                     waits=[("C", sevac_v[q]), ("V", scale_v)], inc="V")
            ow_v = cnt["V"]
            emit("SP", lambda: nc.sync.dma_start(out=oscr[:], in_=owst),
                 waits=[("V", ow_v)], inc="O", inc_by=16)
            o1 = cnt["O"]
            gap = BassAP(oscr[:].tensor, 0, [[8256, 32], [2048, 4], [1, 64]])
            emit("SP",
                 lambda g=gap: nc.sync.dma_start(
                     out=out.rearrange("b k m -> k b m"), in_=g),
                 waits=[("O", o1)], inc="O", inc_by=16)
            emit("SP", lambda: None, waits=[("O", cnt["O"])])

    # ---- emission: one body per engine ----
    with (
        nc.semaphore("sP") as sP,
        nc.semaphore("sV") as sV,
        nc.semaphore("sC") as sC,
        nc.semaphore("sL") as sL,
        nc.semaphore("sO") as sO,
        nc.semaphore("sX0") as sX0,
        nc.semaphore("sX1") as sX1,
        nc.semaphore("sX2") as sX2,
        nc.semaphore("sX3") as sX3,
        nc.semaphore("sT0") as sT0,
        nc.semaphore("sT1") as sT1,
        nc.semaphore("sT2") as sT2,
        nc.semaphore("sT3") as sT3,
        nc.semaphore("sW1") as sW1,
        nc.semaphore("sW2") as sW2,
        nc.semaphore("sM1") as sM1,
        nc.allow_low_precision(reason="routing softmax/sumsq tolerate bf16"),
        nc.Block() as block,
    ):
        sem_handles = {"P": sP, "V": sV, "C": sC, "L": sL, "O": sO,
                       "X0": sX0, "X1": sX1, "X2": sX2, "X3": sX3,
                       "T0": sT0, "T1": sT1, "T2": sT2, "T3": sT3,
                       "W1": sW1, "W2": sW2, "M1": sM1}

        def run_ops(eng_name):
            def body(e):
                for waits, fn, inc, inc_by in ops[eng_name]:
                    for sem_key, val in waits:
                        e.wait_ge(sem_handles[sem_key], val)
                    inst = fn()
                    if inc is not None and inst is not None:
                        inst.then_inc(sem_handles[inc], inc_by)
            return body

        block.sync(run_ops("SP"))
        block.tensor(run_ops("PE"))
        block.scalar(run_ops("ACT"))
        block.vector(run_ops("DVE"))

    return nc


def _get_nc():
    if "nc" not in _CACHED:
        _CACHED["nc"] = _build_nc()
    return _CACHED["nc"]


def kernel(x, W):
    global LAST_EXEC_NS
    import ml_dtypes
    from concourse.bass_utils import run_bass_kernel_spmd

    bf16 = ml_dtypes.bfloat16
    x = np.ascontiguousarray(x, dtype=np.float32)
    W = np.ascontiguousarray(W, dtype=np.float32)
    assert x.shape == (B_FULL, N, D) and W.shape == (D, KM)

    nc = _get_nc()

    wbf = W.astype(bf16)
    wtbf = np.ascontiguousarray(W.T).astype(bf16)
    cc0 = np.full((128, K), 1.0 / K, dtype=bf16)
    idb = np.eye(128, dtype=bf16)
    msk8 = np.zeros((128, 8), dtype=bf16)
    msk8[0:64, 0:4] = 1.0
    msk8[64:128, 4:8] = 1.0
    msk32 = np.zeros((128, K), dtype=bf16)
    for p in range(128):
        msk32[p, p // BPC] = 1.0

    in_maps = []
    for i in range(NCORES):
        xs = x[i * BPC:(i + 1) * BPC]
        xbf = np.ascontiguousarray(xs).astype(bf16)
        xtbf = np.ascontiguousarray(xs.transpose(0, 2, 1)).astype(bf16)
        in_maps.append(
            {"xb": xbf, "xtb": xtbf, "wb": wbf, "wtb": wtbf, "cc0": cc0,
             "idb": idb, "msk8": msk8, "msk32": msk32}
        )

    trace = os.environ.get("KERNEL_TRACE", "0") == "1"
    res = run_bass_kernel_spmd(nc, in_maps, list(range(NCORES)), trace=trace)
    LAST_EXEC_NS = res.exec_time_ns
    outs = [res.results[i]["out"] for i in range(NCORES)]
    return np.concatenate(outs, axis=0)
